# revision 1
# baseline (speedup 1.0000x reference)
"""Trainium2 Bass kernel for CausalMessagePassingLayer (GCN message passing).

Contract: kernel(**inputs) takes FULL unsharded inputs (numpy), returns the
FULL output. Internally shards batch B=16 across 8 NeuronCores (2 samples per
core), builds one SPMD Bass/Tile program, and runs it via
run_bass_kernel_spmd.

Math (per sample), with x = t_emb[t2e], A-hat = binary adjacency incl.
self-loops, dinv = 1/sqrt(deg):
    y0   = dinv * x                       (row scale)
    agg  = A_hat @ y0                     (message aggregation, binary one-hot
                                           matmuls over col-sorted messages)
    z    = dinv * agg                     (row scale)
    gnn  = z @ W.T + b
    causal[j] = gnn[j-1], causal[0] = 0   (folded into scatter indices)
    out  = t_emb;  out[e2t[j]] += causal[j]
"""
import os
import numpy as np
from contextlib import ExitStack

import concourse.bacc as bacc
import concourse.mybir as mybir
from concourse import tile, library_config
from concourse.bass_utils import run_bass_kernel_spmd

F32 = mybir.dt.float32
BF16 = mybir.dt.bfloat16
I16 = mybir.dt.int16
BF16_NP = mybir.dt.np(BF16)

B, S, D, E, M = 16, 8192, 256, 4096, 32768
NCORES, SPC = 8, 2          # cores, samples per core
NCT = E // 128              # 32 c-tiles per sample
KH = D // 128               # 2 contraction halves


def _wrap_idx(ix):
    """Wrapped SWDGE index layout: [128, n//16]; idx i at [i%16, i//16],
    replicated across the 8 Q7 cores (16-partition groups)."""
    n = ix.shape[0]
    w = ix.reshape(n // 16, 16).T.astype(np.int16)
    return np.tile(w, (8, 1))


def _prep_sample(row, col, t2e, e2t, bpc):
    """Host-side index preprocessing for one sample."""
    deg = 1.0 + np.bincount(col, minlength=E)
    dinv = (1.0 / np.sqrt(deg)).astype(np.float32)

    rows_all = np.concatenate([row, np.arange(E)])   # self-loops as messages
    cols_all = np.concatenate([col, np.arange(E)])
    # group by c-tile (for the P-phase), row-sorted within each tile so the
    # message gather reads HBM in ascending address order (row-buffer hits)
    order = np.lexsort((rows_all, cols_all >> 7))
    r_s, c_s = rows_all[order], cols_all[order]

    cnt = np.bincount(c_s >> 7, minlength=NCT)       # messages per c-tile
    npad_ct = bpc * 128
    rows_padded = np.zeros((NCT, npad_ct), np.int64)
    clocal = np.full((NCT, npad_ct), -1.0, np.float32)
    offs = np.concatenate([[0], np.cumsum(cnt)])
    for t in range(NCT):
        n = cnt[t]
        rows_padded[t, :n] = r_s[offs[t] : offs[t + 1]]
        clocal[t, :n] = c_s[offs[t] : offs[t + 1]] & 127

    nblk = NCT * bpc
    rows_w = _wrap_idx(rows_padded.reshape(-1))                  # [128, NPAD//16]
    cloc = clocal.reshape(nblk, 128).T.copy()                    # [128, NBLK] f32
    dinv_t = dinv.reshape(NCT, 128).T.copy()                     # [128, 32]
    t2e_w = _wrap_idx(np.asarray(t2e))                           # [128, 256]
    scat = np.concatenate([np.asarray(e2t)[1:], [-1]])
    scat_w = _wrap_idx(scat)                                     # [128, 256]
    return rows_w, cloc, dinv_t, t2e_w, scat_w


KSTAGE = os.environ.get("KSTAGE", "full")  # debug bisect: gath|pphase|trans|full
KREPEAT = int(os.environ.get("KREPEAT", "1"))  # timing: repeat whole pipeline


def _build_program(bpc):
    """Build the SPMD Bass program (one core's view: SPC samples).

    All SWDGE gather/scatter instructions are chunked to <=1024 indices —
    a single instruction above the SWDGE descriptor-ring capacity hangs on
    hardware (2048 fails, 1024 passes)."""
    nblk = NCT * bpc
    npad = nblk * 128
    ch_blocks = 8                             # blocks per gather chunk (1024 idxs)
    ch_idx = ch_blocks * 128
    nch = (nblk + ch_blocks - 1) // ch_blocks # msg gather chunks per sample

    nc = bacc.Bacc("TRN2", target_bir_lowering=False, debug=False)

    t_emb_d = nc.dram_tensor("t_emb", [SPC, S, D], F32, kind="ExternalInput").ap()
    t2e_d = nc.dram_tensor("t2e_w", [SPC, 128, E // 16], I16, kind="ExternalInput").ap()
    rows_d = nc.dram_tensor("rows_w", [SPC, 128, npad // 16], I16, kind="ExternalInput").ap()
    cloc_d = nc.dram_tensor("cloc", [SPC, 128, nblk], F32, kind="ExternalInput").ap()
    scat_d = nc.dram_tensor("scat_w", [SPC, 128, E // 16], I16, kind="ExternalInput").ap()
    dinv_d = nc.dram_tensor("dinv_t", [SPC, 128, NCT], F32, kind="ExternalInput").ap()
    wt_d = nc.dram_tensor("wt", [KH, 128, D], BF16, kind="ExternalInput").ap()
    b_d = nc.dram_tensor("b_bc", [128, D], F32, kind="ExternalInput").ap()
    iota_d = nc.dram_tensor("iota_bf", [128, 128], BF16, kind="ExternalInput").ap()
    id_d = nc.dram_tensor("id_bf", [128, 128], BF16, kind="ExternalInput").ap()
    out_d = nc.dram_tensor("out", [SPC, S, D], F32, kind="ExternalOutput").ap()
    y0_d = nc.dram_tensor("y0_hbm", [SPC, E, D], BF16, kind="Internal").ap()

    with tile.TileContext(nc) as tc, ExitStack() as ctx:
        nc.gpsimd.load_library(library_config.mlp)

        cpool = ctx.enter_context(tc.tile_pool(name="const", bufs=1))
        meta = ctx.enter_context(tc.tile_pool(name="meta", bufs=2))
        epool = ctx.enter_context(tc.tile_pool(name="edge", bufs=1))
        ypool = ctx.enter_context(tc.tile_pool(name="y0", bufs=1))
        mpool = ctx.enter_context(tc.tile_pool(name="msg", bufs=3))
        ppool = ctx.enter_context(tc.tile_pool(name="pblk", bufs=4))
        zpool = ctx.enter_context(tc.tile_pool(name="z", bufs=1))
        ztpool = ctx.enter_context(tc.tile_pool(name="zt", bufs=1))
        capool = ctx.enter_context(tc.tile_pool(name="causal", bufs=2))
        ps_p = ctx.enter_context(tc.tile_pool(name="ps_p", bufs=3, space="PSUM"))
        ps_t = ctx.enter_context(tc.tile_pool(name="ps_t", bufs=2, space="PSUM"))
        ps_f = ctx.enter_context(tc.tile_pool(name="ps_f", bufs=2, space="PSUM"))

        # constants
        wt_sb = cpool.tile([128, KH, D], BF16)
        for kh in range(KH):
            nc.sync.dma_start(wt_sb[:, kh, :], wt_d[kh])
        b_sb = cpool.tile([128, D], F32)
        nc.sync.dma_start(b_sb[:], b_d[:])
        iota_sb = cpool.tile([128, 128], BF16)
        nc.sync.dma_start(iota_sb[:], iota_d[:])
        id_sb = cpool.tile([128, 128], BF16)
        nc.sync.dma_start(id_sb[:], id_d[:])

        for _rep in range(KREPEAT):
            deferred_scatters = []
            for s in range(SPC):
                # --- metadata loads
                t2e_sb = meta.tile([128, E // 16], I16, tag="t2e")
                nc.sync.dma_start(t2e_sb[:], t2e_d[s])
                rows_sb = meta.tile([128, npad // 16], I16, tag="rows")
                nc.sync.dma_start(rows_sb[:], rows_d[s])
                cloc_sb = meta.tile([128, nblk], F32, tag="cloc")
                nc.sync.dma_start(cloc_sb[:], cloc_d[s])
                scat_sb = meta.tile([128, E // 16], I16, tag="scat")
                nc.sync.dma_start(scat_sb[:], scat_d[s])
                dinv_sb = meta.tile([128, NCT], F32, tag="dinv")
                nc.sync.dma_start(dinv_sb[:], dinv_d[s])

                # --- edge gather: x = t_emb[t2e]  -> [128, 32, 256] f32
                edge_sb = epool.tile([128, NCT, D], F32)
                for c in range(E // 1024):
                    nc.gpsimd.dma_gather(
                        edge_sb[:, c * 8 : (c + 1) * 8, :], t_emb_d[s],
                        t2e_sb[:, c * 64 : (c + 1) * 64], 1024, 1024, D,
                    )

                # --- y0 = dinv * x -> bf16, then to HBM
                y0_sb = ypool.tile([128, NCT, D], BF16)
                for cb in range(NCT):
                    nc.vector.tensor_scalar(
                        y0_sb[:, cb, :], edge_sb[:, cb, :],
                        dinv_sb[:, cb : cb + 1], None, op0=mybir.AluOpType.mult,
                    )
                nc.sync.dma_start(
                    y0_d[s].rearrange("(cb p) d -> p cb d", p=128), y0_sb[:]
                )

                # --- message gather chunks (8 blocks = 1024 idxs each) + P-phase
                z_sb = zpool.tile([128, NCT, D], BF16)
                if KSTAGE == "gath":
                    # anchor gathers without P-phase: copy msgs into z cheaply
                    for ch in range(nch):
                        t = mpool.tile([128, ch_blocks, D], BF16, tag="msg")
                        nc.gpsimd.dma_gather(
                            t[:], y0_d[s],
                            rows_sb[:, ch * (ch_idx // 16) : (ch + 1) * (ch_idx // 16)],
                            ch_idx, ch_idx, D)
                        if ch < NCT:
                            nc.vector.tensor_copy(z_sb[:, ch % NCT, :], t[:, 0, :])
                    zf = capool.tile([128, NCT, D], F32, tag="zf")
                    nc.vector.tensor_copy(zf[:], z_sb[:])
                    nc.sync.dma_start(out_d[s][:E].rearrange("(cb p) d -> p cb d", p=128), zf[:])
                    continue
                msg_tiles = {}

                def get_msg(ch):
                    if ch not in msg_tiles:
                        t = mpool.tile([128, ch_blocks, D], BF16, tag="msg")
                        nc.gpsimd.dma_gather(
                            t[:], y0_d[s],
                            rows_sb[:, ch * (ch_idx // 16) : (ch + 1) * (ch_idx // 16)],
                            ch_idx, ch_idx, D,
                        )
                        msg_tiles[ch] = t
                    return msg_tiles[ch]

                for ct in range(NCT):
                    ps = ps_p.tile([128, D], F32, tag="agg")
                    for j in range(bpc):
                        blk = ct * bpc + j
                        ch, sl = divmod(blk, ch_blocks)
                        msg_sb = get_msg(ch)
                        p_bf = ppool.tile([128, 128], BF16, tag="p")
                        nc.vector.tensor_scalar(
                            p_bf[:], iota_sb[:], cloc_sb[:, blk : blk + 1],
                            None, op0=mybir.AluOpType.is_equal,
                        )
                        nc.tensor.matmul(
                            ps[:], p_bf[:], msg_sb[:, sl, :],
                            start=(j == 0), stop=(j == bpc - 1),
                        )
                    # z = dinv[c] * agg  (psum f32 -> sbuf bf16)
                    nc.vector.tensor_scalar(
                        z_sb[:, ct, :], ps[:], dinv_sb[:, ct : ct + 1],
                        None, op0=mybir.AluOpType.mult,
                    )

                if KSTAGE == "pphase":
                    zf = capool.tile([128, NCT, D], F32, tag="zf")
                    nc.vector.tensor_copy(zf[:], z_sb[:])
                    nc.sync.dma_start(out_d[s][:E].rearrange("(cb p) d -> p cb d", p=128), zf[:])
                    continue

                # --- transpose z -> z_T [2][128, 4096] bf16
                zt_sb = ztpool.tile([128, KH, E], BF16)
                for ct in range(NCT):
                    for kh in range(KH):
                        pst = ps_t.tile([128, 128], BF16, tag="tr")
                        nc.tensor.transpose(
                            pst[:], z_sb[:, ct, kh * 128 : (kh + 1) * 128], id_sb[:]
                        )
                        nc.scalar.copy(
                            zt_sb[:, kh, ct * 128 : (ct + 1) * 128], pst[:]
                        )

                # --- gnn = z @ W.T + b  -> causal slots [128, 32, 256] f32
                causal_sb = capool.tile([128, NCT, D], F32)
                for ec in range(NCT):
                    ps2 = ps_f.tile([128, D], F32, tag="mm")
                    for kh in range(KH):
                        nc.tensor.matmul(
                            ps2[:],
                            zt_sb[:, kh, ec * 128 : (ec + 1) * 128],
                            wt_sb[:, kh, :],
                            start=(kh == 0), stop=(kh == KH - 1),
                        )
                    nc.vector.tensor_tensor(
                        causal_sb[:, ec, :], ps2[:], b_sb[:], op=mybir.AluOpType.add
                    )

                if KSTAGE == "trans":
                    nc.sync.dma_start(out_d[s][:E].rearrange("(cb p) d -> p cb d", p=128), causal_sb[:])
                    continue

                # --- output base copy; scatters deferred behind a barrier (the
                # copy+scatter+pipeline concurrency crashed the exec unit on HW)
                if KSTAGE != "nocopy":
                    nc.sync.dma_start(out_d[s], t_emb_d[s])
                if KSTAGE == "noscat":
                    continue
                deferred_scatters.append((s, causal_sb, scat_sb))

            # --- all scatter-adds after everything else has drained
            if deferred_scatters:
                tc.strict_bb_all_engine_barrier()
                for s, causal_sb, scat_sb in deferred_scatters:
                    for c in range(E // 1024):
                        nreg = 1024 if c < E // 1024 - 1 else 1023  # trailing -1 pad
                        nc.gpsimd.dma_scatter_add(
                            out_d[s], causal_sb[:, c * 8 : (c + 1) * 8, :],
                            scat_sb[:, c * 64 : (c + 1) * 64], 1024, nreg, D,
                        )

    nc.compile()
    return nc


def kernel(token_embeddings, tokens2edges, edge_index, edges2tokens, W, b):
    token_embeddings = np.ascontiguousarray(np.asarray(token_embeddings, dtype=np.float32))
    tokens2edges = np.asarray(tokens2edges)
    edge_index = np.asarray(edge_index)
    edges2tokens = np.asarray(edges2tokens)
    W = np.asarray(W, dtype=np.float32)
    b = np.asarray(b, dtype=np.float32)

    # global uniform blocks-per-ctile so all cores share one program
    bpc = 0
    for bi in range(B):
        col = edge_index[bi, 1].astype(np.int64)
        cnt = np.bincount(
            np.concatenate([col, np.arange(E)]) >> 7, minlength=NCT
        )
        bpc = max(bpc, int(np.max((cnt + 127) // 128)))

    preps = [
        _prep_sample(
            edge_index[bi, 0].astype(np.int64),
            edge_index[bi, 1].astype(np.int64),
            tokens2edges[bi], edges2tokens[bi], bpc,
        )
        for bi in range(B)
    ]

    wt_host = np.ascontiguousarray(W.T).astype(BF16_NP).reshape(KH, 128, D)
    b_bc = np.tile(b[None, :], (128, 1)).astype(np.float32)
    iota_bf = np.tile(np.arange(128, dtype=np.float32)[None, :], (128, 1)).astype(BF16_NP)
    id_bf = np.eye(128, dtype=np.float32).astype(BF16_NP)

    nc = _build_program(bpc)

    in_maps = []
    for c in range(NCORES):
        sl = slice(c * SPC, (c + 1) * SPC)
        rows_w = np.stack([preps[bi][0] for bi in range(sl.start, sl.stop)])
        cloc = np.stack([preps[bi][1] for bi in range(sl.start, sl.stop)])
        dinv_t = np.stack([preps[bi][2] for bi in range(sl.start, sl.stop)])
        t2e_w = np.stack([preps[bi][3] for bi in range(sl.start, sl.stop)])
        scat_w = np.stack([preps[bi][4] for bi in range(sl.start, sl.stop)])
        in_maps.append({
            "t_emb": np.ascontiguousarray(token_embeddings[sl]),
            "t2e_w": t2e_w, "rows_w": rows_w, "cloc": cloc,
            "scat_w": scat_w, "dinv_t": dinv_t,
            "wt": wt_host, "b_bc": b_bc, "iota_bf": iota_bf, "id_bf": id_bf,
        })

    res = run_bass_kernel_spmd(nc, in_maps, list(range(NCORES)))
    out = np.concatenate([r["out"] for r in res.results], axis=0)
    return out.astype(np.float32)



# revision 2
# speedup vs baseline: 3.5143x; 3.5143x over previous
"""Trainium2 Bass kernel for CausalMessagePassingLayer — instruction-minimal v2.

This hardware dispatches ~1 instruction per ~50us globally (all engines
serialized), so the ONLY thing that matters is total instruction count.
Design: a handful of fat instructions per stage.

Math per sample (x = t_emb[t2e], A-hat incl self-loops, dinv = deg^-1/2):
    y0   = dinv * x                       f32
    agg  = A_hat @ y0                     (64 chained SWDGE scatter-add chunks,
                                           unique targets per chunk; self-loop
                                           via agg init = y0)
    z    = dinv * agg                     (transposed layout, bf16)
    gnn  = z @ W.T                        (16 fat matmuls/sample, free dim 512)
    out  = out_init + scatter(causal)     (out_init = t_emb + b at targets,
                                           donated as the output buffer)
"""
import os
import numpy as np
from contextlib import ExitStack

import concourse.bacc as bacc
import concourse.mybir as mybir
from concourse import tile, library_config
from concourse.bass_utils import run_bass_kernel_spmd

F32 = mybir.dt.float32
BF16 = mybir.dt.bfloat16
I16 = mybir.dt.int16
BF16_NP = mybir.dt.np(BF16)

B, S, D, E, M = 16, 8192, 256, 4096, 32768
NCORES, SPC = 8, 2
NCT = E // 128               # 32 e-tiles per sample
NMSG_ALL = SPC * M           # 65536 messages per core
NCH = NMSG_ALL // 1024       # 64 scatter chunks

KSTAGE = os.environ.get("KSTAGE", "full")
KREPEAT = int(os.environ.get("KREPEAT", "1"))


def _wrap_idx(ix):
    """SWDGE index layout: [128, n//16]; idx i at [i%16, i//16], tiled to 8 Q7 cores."""
    n = ix.shape[0]
    w = ix.reshape(n // 16, 16).T.astype(np.int16)
    return np.tile(w, (8, 1))


def _build_program():
    nc = bacc.Bacc("TRN2", target_bir_lowering=False, debug=False)

    t_emb_d = nc.dram_tensor("t_emb", [SPC * S, D], F32, kind="ExternalInput").ap()
    t2e_d = nc.dram_tensor("t2e_w", [128, SPC * E // 16], I16, kind="ExternalInput").ap()
    rows_d = nc.dram_tensor("rows_w", [128, NMSG_ALL // 16], I16, kind="ExternalInput").ap()
    cols_d = nc.dram_tensor("cols_w", [128, NMSG_ALL // 16], I16, kind="ExternalInput").ap()
    scat_d = nc.dram_tensor("scat_w", [128, SPC * E // 16], I16, kind="ExternalInput").ap()
    dinv_d = nc.dram_tensor("dinv_bc", [128, SPC * NCT, D], F32, kind="ExternalInput").ap()
    dinvt_d = nc.dram_tensor("dinv_bct", [128, SPC * 2, E], BF16, kind="ExternalInput").ap()
    wt_d = nc.dram_tensor("wt", [2, 2, 128, 128], BF16, kind="ExternalInput").ap()
    out_d = nc.dram_tensor("out", [SPC * S, D], F32, kind="ExternalOutput").ap()

    y0_dram = nc.dram_tensor("y0_hbm", [SPC * E, D], F32, kind="Internal").ap()
    agg_dram = nc.dram_tensor("agg_hbm", [SPC * E, D], F32, kind="Internal").ap()
    causal_dram = nc.dram_tensor("causal_hbm", [SPC * E, D], F32, kind="Internal").ap()

    with tile.TileContext(nc) as tc, ExitStack() as ctx:
        nc.gpsimd.load_library(library_config.mlp)

        cpool = ctx.enter_context(tc.tile_pool(name="const", bufs=1))
        # persistent small metadata (~3KB/partition)
        t2e_sb = cpool.tile([128, SPC * E // 16], I16)
        nc.sync.dma_start(t2e_sb[:], t2e_d[:])
        scat_sb = cpool.tile([128, SPC * E // 16], I16)
        nc.sync.dma_start(scat_sb[:], scat_d[:])
        wt_sb = cpool.tile([128, 2, 2, 128], BF16)
        nc.sync.dma_start(wt_sb[:], wt_d.rearrange("j k p m -> p j k m"))

        for _rep in range(KREPEAT):
            with tc.tile_pool(name="big1", bufs=1) as big1:
                # --- x gather:  x[p, s*32+cb, :] = t_emb[t2e[...]]
                x_sb = big1.tile([128, SPC * NCT, D], F32, tag="big1")
                for c in range(SPC * E // 1024):
                    nc.gpsimd.dma_gather(
                        x_sb[:, c * 8 : (c + 1) * 8, :], t_emb_d,
                        t2e_sb[:, c * 64 : (c + 1) * 64], 1024, 1024, D,
                    )

                # --- y0 = dinv * x  (in place)
                with tc.tile_pool(name="dinvp", bufs=1) as dinvp:
                    dinv_sb = dinvp.tile([128, SPC * NCT, D], F32, tag="dinv")
                    nc.sync.dma_start(dinv_sb[:], dinv_d[:])
                    nc.vector.tensor_tensor(
                        x_sb[:], x_sb[:], dinv_sb[:], op=mybir.AluOpType.mult)

                # --- y0 -> DRAM; agg init = y0 (self-loops)
                nc.sync.dma_start(
                    y0_dram.rearrange("(b p) d -> p b d", p=128), x_sb[:])
                nc.sync.dma_start(agg_dram, y0_dram)

                if KSTAGE == "gath":
                    nc.sync.dma_start(out_d[: SPC * E], y0_dram)
                    continue

                # --- message chain: 64 x (gather 1024 + scatter-add 1024 unique)
                if KSTAGE != "nomsg":
                    with tc.tile_pool(name="msgidx", bufs=1) as msgidx, \
                         tc.tile_pool(name="msgp", bufs=2) as msgp:
                        rows_sb = msgidx.tile([128, NMSG_ALL // 16], I16, tag="rw")
                        nc.sync.dma_start(rows_sb[:], rows_d[:])
                        cols_sb = msgidx.tile([128, NMSG_ALL // 16], I16, tag="cw")
                        nc.sync.dma_start(cols_sb[:], cols_d[:])
                        for ch in range(NCH):
                            msg_sb = msgp.tile([128, 8, D], F32, tag="msg")
                            nc.gpsimd.dma_gather(
                                msg_sb[:], y0_dram,
                                rows_sb[:, ch * 64 : (ch + 1) * 64], 1024, 1024, D,
                            )
                            nc.gpsimd.dma_scatter_add(
                                agg_dram, msg_sb[:],
                                cols_sb[:, ch * 64 : (ch + 1) * 64], 1024, 1024, D,
                            )

                if KSTAGE == "msg":
                    nc.sync.dma_start(out_d[: SPC * E], agg_dram)
                    continue

                # --- zT load, scale, matmul; psum drains straight to causal_dram
                ca_r = causal_dram.rearrange("(s e) (j p) -> p s j e", p=128, s=SPC)
                with tc.tile_pool(name="ztp", bufs=1) as ztp, \
                     tc.tile_pool(name="dtp", bufs=1) as dtp, \
                     tc.tile_pool(name="ztbp", bufs=1) as ztbp, \
                     tc.tile_pool(name="psp", bufs=4, space="PSUM") as psp:
                    zt_sb = ztp.tile([128, SPC * 2, E], F32, tag="zt")
                    agg_r = agg_dram.rearrange("(s e) (j p) -> p s j e", p=128, s=SPC)
                    for s in range(SPC):
                        for j in range(2):
                            nc.sync.dma_start(zt_sb[:, s * 2 + j, :], agg_r[:, s, j, :])

                    dinvt_sb = dtp.tile([128, SPC * 2, E], BF16, tag="dt")
                    nc.sync.dma_start(dinvt_sb[:], dinvt_d[:])
                    ztb_sb = ztbp.tile([128, SPC * 2, E], BF16, tag="ztb")
                    nc.vector.tensor_tensor(
                        ztb_sb[:], zt_sb[:], dinvt_sb[:], op=mybir.AluOpType.mult)

                    causal_t = ztp.tile([128, SPC * 2, E], F32, tag="zt")
                    for s in range(SPC):
                        for j in range(2):
                            for eo in range(E // 1024):
                                ps = psp.tile([128, 1024], F32, tag="mm")
                                for half in range(2):
                                    sl = slice((eo * 2 + half) * 512,
                                               (eo * 2 + half + 1) * 512)
                                    pslice = ps[:, half * 512 : (half + 1) * 512]
                                    for k in range(2):
                                        nc.tensor.matmul(
                                            pslice, wt_sb[:, j, k, :],
                                            ztb_sb[:, s * 2 + k, sl],
                                            start=(k == 0), stop=(k == 1),
                                        )
                                nc.scalar.copy(
                                    causal_t[:, s * 2 + j, eo * 1024 : (eo + 1) * 1024],
                                    ps[:])
                    for s in range(SPC):
                        for j in range(2):
                            nc.sync.dma_start(ca_r[:, s, j, :], causal_t[:, s * 2 + j, :])

                if KSTAGE == "mm":
                    nc.sync.dma_start(out_d[: SPC * E], causal_dram)
                    continue

                # --- reload causal std; final scatter into donated out
                causal_sb = big1.tile([128, SPC * NCT, D], F32, tag="big1")
                nc.sync.dma_start(
                    causal_sb[:], causal_dram.rearrange("(b p) d -> p b d", p=128))
                for c in range(SPC * E // 1024):
                    nreg = 1023 if (c % (E // 1024) == E // 1024 - 1) else 1024
                    nc.gpsimd.dma_scatter_add(
                        out_d, causal_sb[:, c * 8 : (c + 1) * 8, :],
                        scat_sb[:, c * 64 : (c + 1) * 64], 1024, nreg, D,
                    )

    nc.compile()
    return nc


def _pack_chunks(cols_all, samp_all):
    """Pack message indices into chunks of 1024 with unique (sample,col) targets
    per chunk. Returns permutation order -> chunk-major message order."""
    key = samp_all.astype(np.int64) * E + cols_all.astype(np.int64)
    order0 = np.argsort(key, kind="stable")
    sorted_key = key[order0]
    # occurrence rank within each (s,col) group
    grp_start = np.r_[0, np.flatnonzero(np.diff(sorted_key)) + 1]
    sizes = np.diff(np.r_[grp_start, len(sorted_key)])
    rank = np.arange(len(sorted_key)) - np.repeat(grp_start, sizes)
    nch = max(NCH, int(sizes.max()))
    # greedy: group i's k-th message -> chunk (start_i + k) % nch, start chosen
    # round-robin by group order; loads tracked to keep <= 1024
    loads = np.zeros(nch, np.int64)
    start = np.zeros(len(sizes), np.int64)
    ptr = 0
    cap = (len(sorted_key) + nch - 1) // nch
    # process biggest groups first for balance
    gorder = np.argsort(-sizes, kind="stable")
    for gi in gorder:
        g = sizes[gi]
        # find start so that chunks ptr..ptr+g-1 all have load < cap..
        tries = 0
        while True:
            cand = [(ptr + k) % nch for k in range(g)]
            if all(loads[c] < 1024 for c in cand):
                break
            ptr = (ptr + 1) % nch
            tries += 1
            assert tries <= nch, "packing failed"
        start[gi] = ptr
        for c in cand:
            loads[c] += 1
        ptr = (ptr + 1) % nch
    chunk_of = (start[np.repeat(np.arange(len(sizes)), sizes)] + rank) % nch
    # message order: chunk-major
    corder = np.argsort(chunk_of, kind="stable")
    perm = order0[corder]
    chunk_sizes = np.bincount(chunk_of, minlength=nch)
    assert chunk_sizes.max() <= 1024
    return perm, chunk_sizes, nch


def _prep_core(edge_index, tokens2edges, edges2tokens, t_emb, b):
    """Host prep for one core's SPC samples. Returns input map (minus consts)."""
    # message list across both samples
    rows_l, cols_l, samp_l = [], [], []
    dinvs = []
    for s in range(SPC):
        row = edge_index[s, 0].astype(np.int64)
        col = edge_index[s, 1].astype(np.int64)
        rows_l.append(row)
        cols_l.append(col)
        samp_l.append(np.full(M, s, np.int64))
        deg = 1.0 + np.bincount(col, minlength=E)
        dinvs.append((1.0 / np.sqrt(deg)).astype(np.float32))
    rows_all = np.concatenate(rows_l)
    cols_all = np.concatenate(cols_l)
    samp_all = np.concatenate(samp_l)

    perm, chunk_sizes, nch = _pack_chunks(cols_all, samp_all)
    assert nch == NCH and np.all(chunk_sizes == 1024), (nch, chunk_sizes.min(), chunk_sizes.max())
    rows_g = (samp_all * E + rows_all)[perm]
    cols_g = (samp_all * E + cols_all)[perm]

    # x-gather idx: slot (b,p): b = s*NCT+cb, edge e = cb*128+p -> token s*S + t2e[s][e]
    t2e_g = np.empty(SPC * E, np.int64)
    scat_g = np.empty(SPC * E, np.int64)
    dinv_bc = np.empty((128, SPC * NCT, D), np.float32)
    dinv_bct = np.empty((128, SPC * 2, E), np.float32)
    for s in range(SPC):
        e_of_slot = np.arange(E)  # slot i (within sample) -> edge e: i = cb*128+p -> e == i
        # slots are ordered b-major: slot index i_global = b*128 + p, b = s*NCT+cb
        # edge for (p, cb): e = cb*128 + p
        cb = np.arange(E) // 128
        p = np.arange(E) % 128
        e_slot = cb * 128 + p  # == arange
        t2e_g[s * E : (s + 1) * E] = s * S + np.asarray(tokens2edges[s])[e_slot]
        e2t = np.asarray(edges2tokens[s]).astype(np.int64)
        scat_tgt = np.concatenate([e2t[1:], [-1]])  # causal: gnn[e] -> e2t[e+1]
        tgt = scat_tgt[e_slot]
        scat_g[s * E : (s + 1) * E] = np.where(tgt >= 0, s * S + tgt, -1)
        dinv_bc[:, s * NCT : (s + 1) * NCT, :] = np.broadcast_to(
            dinvs[s].reshape(NCT, 128).T[:, :, None], (128, NCT, D))
        dinv_bct[:, s * 2 : (s + 1) * 2, :] = np.broadcast_to(
            dinvs[s][None, None, :], (128, 2, E))

    # out_init = t_emb + b at scattered targets
    out_init = np.ascontiguousarray(t_emb.reshape(SPC * S, D)).astype(np.float32)
    if np.any(b != 0):
        out_init = out_init.copy()
        for s in range(SPC):
            tgts = np.asarray(edges2tokens[s]).astype(np.int64)[1:]
            out_init[s * S + tgts] += b[None, :]

    return {
        "t_emb": np.ascontiguousarray(t_emb.reshape(SPC * S, D), dtype=np.float32),
        "t2e_w": _wrap_idx(t2e_g),
        "rows_w": _wrap_idx(rows_g),
        "cols_w": _wrap_idx(cols_g),
        "scat_w": _wrap_idx(scat_g),
        "dinv_bc": dinv_bc,
        "dinv_bct": dinv_bct.astype(BF16_NP),
    }, out_init


_EXEC_CACHE = {}


def _run_spmd(nc, in_maps, out_init_map):
    """run_bass_via_pjrt clone: donated output buffers carry host-provided
    initial contents (instead of zeros); jitted callable cached across calls."""
    import jax
    from jax.sharding import Mesh, PartitionSpec
    from jax.experimental.shard_map import shard_map
    from concourse import bass2jax as b2j
    import concourse.mybir as mb

    n_cores = len(in_maps)
    key = id(nc)
    if key not in _EXEC_CACHE:
        b2j.install_neuronx_cc_hook()
        partition_name = (
            nc.partition_id_tensor.name if nc.partition_id_tensor else None)
        in_names, out_names, out_avals = [], [], []
        for alloc in nc.m.functions[0].allocations:
            if not isinstance(alloc, mb.MemoryLocationSet):
                continue
            name = alloc.memorylocations[0].name
            if alloc.kind == "ExternalInput":
                if name != partition_name:
                    in_names.append(name)
            elif alloc.kind == "ExternalOutput":
                out_names.append(name)
                out_avals.append(jax.core.ShapedArray(
                    tuple(alloc.tensor_shape), mb.dt.np(alloc.dtype)))
        n_params = len(in_names)
        all_names = in_names + out_names
        if partition_name is not None:
            all_names.append(partition_name)

        def _body(*args):
            operands = list(args)
            if partition_name is not None:
                operands.append(b2j.partition_id_tensor())
            outs = b2j._bass_exec_p.bind(
                *operands,
                out_avals=tuple(out_avals),
                in_names=tuple(all_names),
                out_names=tuple(out_names),
                lowering_input_output_aliases=(),
                sim_require_finite=True,
                sim_require_nnan=True,
                nc=nc,
            )
            return tuple(outs)

        donate = tuple(range(n_params, n_params + len(out_names)))
        devices = jax.devices()[:n_cores]
        mesh = Mesh(np.asarray(devices), ("core",))
        in_specs = (PartitionSpec("core"),) * (n_params + len(out_names))
        out_specs = (PartitionSpec("core"),) * len(out_names)
        sharded = jax.jit(
            shard_map(_body, mesh=mesh, in_specs=in_specs,
                      out_specs=out_specs, check_rep=False),
            donate_argnums=donate, keep_unused=True)
        _EXEC_CACHE[key] = (sharded, in_names, out_names, out_avals)

    sharded, in_names, out_names, out_avals = _EXEC_CACHE[key]
    concat_in = [
        np.concatenate([np.asarray(in_maps[c][nm]) for c in range(n_cores)], axis=0)
        for nm in in_names
    ]
    concat_outs = []
    for i, nm in enumerate(out_names):
        if nm in out_init_map:
            concat_outs.append(np.concatenate(
                [np.asarray(v) for v in out_init_map[nm]], axis=0))
        else:
            z = out_avals[i]
            concat_outs.append(np.zeros((n_cores * z.shape[0], *z.shape[1:]), z.dtype))
    out_arrs = sharded(*concat_in, *concat_outs)
    return {
        nm: np.asarray(out_arrs[i]).reshape(n_cores * out_avals[i].shape[0],
                                            *out_avals[i].shape[1:])
        for i, nm in enumerate(out_names)
    }


_PROG_CACHE = {}


def _get_program():
    key = (KSTAGE, KREPEAT)
    if key not in _PROG_CACHE:
        _PROG_CACHE[key] = _build_program()
    return _PROG_CACHE[key]


def kernel(token_embeddings, tokens2edges, edge_index, edges2tokens, W, b):
    token_embeddings = np.asarray(token_embeddings, dtype=np.float32)
    tokens2edges = np.asarray(tokens2edges)
    edge_index = np.asarray(edge_index)
    edges2tokens = np.asarray(edges2tokens)
    W = np.asarray(W, dtype=np.float32)
    b = np.asarray(b, dtype=np.float32)

    wt = np.ascontiguousarray(W.T).astype(BF16_NP)
    w4 = np.zeros((2, 2, 128, 128), BF16_NP)
    for j in range(2):
        for k in range(2):
            w4[j, k] = wt[k * 128 : (k + 1) * 128, j * 128 : (j + 1) * 128]

    nc = _get_program()

    in_maps = []
    out_inits = []
    for c in range(NCORES):
        sl = slice(c * SPC, (c + 1) * SPC)
        m, out_init = _prep_core(
            edge_index[sl], tokens2edges[sl], edges2tokens[sl],
            token_embeddings[sl], b)
        m["wt"] = w4
        in_maps.append(m)
        out_inits.append(out_init)

    out = _run_spmd(nc, in_maps, {"out": out_inits})
    return out["out"].reshape(B, S, D).astype(np.float32)
